# revision 4
# baseline (speedup 1.0000x reference)
"""Trainium2 Bass kernel for a 2-branch GCN siamese network (protein pairs).

Math per graph b (see reference):
    h  = leaky( A_norm @ (x @ Wg) + bg )        # GCNConv + LeakyReLU
    g  = leaky( mean_n(h) @ Wf + bf )
    xc = concat(g1, g2); 2-layer MLP + sigmoid -> scalar

Sharding: data-parallel over the batch of 8 graphs -> core b handles graph b
entirely (both branches + head) and emits a single scalar.

Device strategy (v2):
  - A_norm is materialized dense (2048x2000, fp8e4m3) on the host, transposed,
    with the symmetric-norm coefficients folded in.
  - MM1 (PE, fp8 DoubleRow): H[n, j] = x @ Wg, PSUM-accumulated over 4
    k-tile pairs; H is cast to fp8 on the PSUM->SBUF copy.
  - MM2 (PE, fp8 DoubleRow): Z^T[j, t] = sum_s H[s, j] * A^T[s, t],
    feature-major, 8 source-tile pairs (H pad rows zeroed).
  - pooling fused into ACT Lrelu (hw slope 0.01 == torch LeakyReLU default):
    accum_out gives sum_t leaky(z + bg) per chunk directly; a single DVE
    reduce over the 4 chunk partials yields the pooled m[j].
  - the Wf matvec (gps) is interleaved into MM2's last-chunk j loop (one j
    behind) so only one 1-col matmul remains on the tail.
  - head MLP uses plain weights + Lrelu ACTs (no (0.01W, 0.99W) pairs).
  - DMA issue is split across the SP and Activation queues; the first wg/xt
    loads are split so the first matmul starts ~9us instead of ~12us.
  - fp8 rounding washes out through the matmul contraction, 2000-node mean
    pool and the head projections: measured end-to-end rel err ~1.7e-4.
"""

import os
import sys

import numpy as np

for _p in ("/opt/trn_rl_repo", "/root/.axon_site/_ro/trn_rl_repo"):
    if os.path.isdir(_p) and _p not in sys.path:
        sys.path.insert(0, _p)

import ml_dtypes

B, N, E, F, D = 8, 2000, 64000, 1024, 128
NT = 2048          # padded node count (sources)
KT = F // 128      # 8 k-tiles over the feature dim
NB = 16            # node blocks for MM1 (15 full + one of 80)
ST = NT // 128     # 16 source tiles for MM2
TC = 4             # target chunks for MM2: widths 512,512,512,464
SLOPE = 0.01

_BF16 = ml_dtypes.bfloat16
_FP8 = ml_dtypes.float8_e4m3

FP8_MM2 = True
FP8_MM1 = True

_NC = None


def _build_program():
    import concourse.bacc as bacc
    import concourse.mybir as mybir
    import concourse.tile as tile

    f32 = mybir.dt.float32
    bf16 = mybir.dt.bfloat16
    AF = mybir.ActivationFunctionType
    AL = mybir.AluOpType
    AX = mybir.AxisListType

    nc = bacc.Bacc()

    def ein(name, shape, dt):
        return nc.dram_tensor(name, shape, dt, kind="ExternalInput")

    f8 = mybir.dt.float8e4
    hdt = f8 if FP8_MM2 else bf16
    xdt = f8 if FP8_MM1 else bf16
    xt_d = [ein("xt1", [F, N], xdt), ein("xt2", [F, N], xdt)]
    wg_d = [ein("wg1", [F, F], xdt), ein("wg2", [F, F], xdt)]
    at_d = [ein("at1", [NT, N], hdt), ein("at2", [NT, N], hdt)]
    bg_d = [ein("bg1", [128, KT], f32), ein("bg2", [128, KT], f32)]
    wf_d = [ein("wf1", [F, D], f32), ein("wf2", [F, D], f32)]
    bf_d = [ein("bf1", [D, 1], f32), ein("bf2", [D, 1], f32)]
    w1_d = ein("w1", [2 * D, 256], f32)
    b1_d = ein("b1", [128, 2], f32)
    w2_d = ein("w2", [256, 64], f32)
    b2_d = ein("b2", [64, 1], f32)
    wo_d = ein("wo", [64, 1], f32)
    bo_d = ein("bo", [1, 1], f32)
    out_d = nc.dram_tensor("out", [1, 1], f32, kind="ExternalOutput")

    cw = [512, 512, 512, 464]            # MM2 chunk widths
    c0 = [0, 512, 1024, 1536]            # chunk column offsets in A^T

    with tile.TileContext(nc) as tc, \
            tc.tile_pool(name="p_xt", bufs=2) as p_xt, \
            tc.tile_pool(name="p_wg", bufs=2) as p_wg, \
            tc.tile_pool(name="p_h", bufs=2) as p_h, \
            tc.tile_pool(name="p_at", bufs=2) as p_at, \
            tc.tile_pool(name="p_c", bufs=1) as p_c, \
            tc.tile_pool(name="p_scr", bufs=3) as p_scr, \
            tc.tile_pool(name="p_vec", bufs=2) as p_vec, \
            tc.tile_pool(name="ps_mm1", bufs=4, space="PSUM") as ps_mm1, \
            tc.tile_pool(name="ps_mm2", bufs=2, space="PSUM") as ps_mm2, \
            tc.tile_pool(name="ps_sm", bufs=2, space="PSUM") as ps_sm:

        # ================= DMA issue order is the critical path =============
        # Two issue queues: SP carries wg1 (split pair 0) + pooled consts +
        # A^T chunks + xt2; ACT carries xt1 (split pair 0) + wg2 + head
        # consts.  The first matmul needs only wg1[kp0,jh0] + xt1[kp0,n<1024].
        wg_sb = [p_wg.tile([128, KT, F], xdt, name=f"wg_sb{br}", tag="wg")
                 for br in range(2)]
        xt_sb = [p_xt.tile([128, KT, N], xdt, name=f"xt_sb{br}", tag="xt")
                 for br in range(2)]
        wgr = [wg_d[br][:, :].rearrange("(kt p) j -> p kt j", p=128)
               for br in range(2)]
        xtr = [xt_d[br][:, :].rearrange("(kt p) n -> p kt n", p=128)
               for br in range(2)]

        bgr_sb = []
        for br in range(2):
            bgr_sb.append(p_c.tile([128, KT], f32, name=f"bgr_sb{br}",
                                   tag=f"bgr{br}"))

        atr = [at_d[br][:, :].rearrange("(so p) t -> p so t", p=128)
               for br in range(2)]
        at_sb = [[p_at.tile([128, ST, 512], hdt,
                            name=f"at_sb{br}_{tcx}", tag="at")
                  for tcx in range(TC)] for br in range(2)]

        def load_at(br, tcx):
            nc.sync.dma_start(
                out=at_sb[br][tcx][:, :, :cw[tcx]],
                in_=atr[br][:, :, c0[tcx]:c0[tcx] + cw[tcx]])

        # --- SP queue ---
        nc.sync.dma_start(out=wg_sb[0][:, 0:2, 0:512],
                          in_=wgr[0][:, 0:2, 0:512])
        nc.sync.dma_start(out=wg_sb[0][:, 0:2, 512:1024],
                          in_=wgr[0][:, 0:2, 512:1024])
        for q in range(1, 4):
            nc.sync.dma_start(out=wg_sb[0][:, 2 * q:2 * q + 2, :],
                              in_=wgr[0][:, 2 * q:2 * q + 2, :])
        nc.sync.dma_start(out=bgr_sb[0][:], in_=bg_d[0][:, :])
        load_at(0, 0)
        load_at(0, 1)
        nc.sync.dma_start(out=bgr_sb[1][:], in_=bg_d[1][:, :])
        b1_sb = p_c.tile([128, 2], f32, name="b1_sb", tag="b1")
        nc.sync.dma_start(out=b1_sb[:], in_=b1_d[:, :])
        b2_sb = p_c.tile([64, 1], f32, name="b2_sb", tag="b2")
        nc.sync.dma_start(out=b2_sb[:], in_=b2_d[:, :])
        bo_sb = p_c.tile([1, 1], f32, name="bo_sb", tag="bo")
        nc.sync.dma_start(out=bo_sb[:], in_=bo_d[:, :])
        for q in range(4):
            nc.sync.dma_start(out=xt_sb[1][:, 2 * q:2 * q + 2, :],
                              in_=xtr[1][:, 2 * q:2 * q + 2, :])

        # --- ACT queue ---
        nc.scalar.dma_start(out=xt_sb[0][:, 0:2, 0:1024],
                            in_=xtr[0][:, 0:2, 0:1024])
        nc.scalar.dma_start(out=xt_sb[0][:, 0:2, 1024:2000],
                            in_=xtr[0][:, 0:2, 1024:2000])
        for q in range(1, 4):
            nc.scalar.dma_start(out=xt_sb[0][:, 2 * q:2 * q + 2, :],
                                in_=xtr[0][:, 2 * q:2 * q + 2, :])
        for h in range(2):
            nc.scalar.dma_start(out=wg_sb[1][:, 4 * h:4 * h + 4, :],
                                in_=wgr[1][:, 4 * h:4 * h + 4, :])
        wf_sb, bf_sb = [], []
        for br in range(2):
            wf_t = p_c.tile([128, KT, D], f32, name=f"wf_sb{br}", tag=f"wf{br}")
            nc.scalar.dma_start(
                out=wf_t[:], in_=wf_d[br][:, :].rearrange("(kt p) d -> p kt d", p=128))
            wf_sb.append(wf_t)
            bf_t = p_c.tile([D, 1], f32, name=f"bf_sb{br}", tag=f"bf{br}")
            nc.scalar.dma_start(out=bf_t[:], in_=bf_d[br][:, :])
            bf_sb.append(bf_t)
        w1_sb = p_c.tile([128, 2, 256], f32, name="w1_sb", tag="w1")
        nc.scalar.dma_start(
            out=w1_sb[:], in_=w1_d[:, :].rearrange("(kt p) m -> p kt m", p=128))
        w2_sb = p_c.tile([128, 2, 64], f32, name="w2_sb", tag="w2")
        nc.scalar.dma_start(
            out=w2_sb[:], in_=w2_d[:, :].rearrange("(kt p) m -> p kt m", p=128))
        wo_sb = p_c.tile([64, 1], f32, name="wo_sb", tag="wo")
        nc.scalar.dma_start(out=wo_sb[:], in_=wo_d[:, :])

        # warm the ACT tables (Sigmoid set; Lrelu rides in every set)
        sigwarm = p_vec.tile([1, 2], f32, name="sigwarm", tag="sigwarm")
        nc.scalar.activation(out=sigwarm[:, 0:1], in_=bo_sb, func=AF.Sigmoid)
        nc.scalar.activation(out=sigwarm[:, 1:2], in_=bo_sb, func=AF.Lrelu,
                             alpha=SLOPE)

        # ========================== compute ================================
        g_vec = []
        m_sb = [None, None]
        gps = [None, None]

        def issue_gps(br, kt):
            nc.tensor.matmul(gps[br], lhsT=wf_sb[br][:, kt, :],
                             rhs=m_sb[br][:, kt:kt + 1],
                             start=(kt == 0), stop=(kt == KT - 1))

        for br in range(2):
            h_sb = p_h.tile([128, ST, F], hdt, name=f"h_sb{br}", tag="h")
            if FP8_MM2:
                # DoubleRow pairs contract the full 16 s-tiles, so H's pad
                # rows (2000..2047) must be zero, not garbage.
                nc.vector.memset(h_sb[64:128, 15, :], 0.0)

            # ---- MM1: H[n, j] = x @ Wg ----
            for nb in range(NB):
                n0 = nb * 128
                m = min(128, N - n0)
                pt = [ps_mm1.tile([128, 512], mybir.dt.float32,
                                  name=f"mm1ps_{br}_{nb}_{jh}", tag="mm1ps")
                      for jh in range(2)]
                for kp in range(KT // 2):
                    for jh in range(2):
                        nc.tensor.matmul(
                            pt[jh][:m, :],
                            lhsT=xt_sb[br][:, 2 * kp:2 * kp + 2, n0:n0 + m],
                            rhs=wg_sb[br][:, 2 * kp:2 * kp + 2,
                                          jh * 512:(jh + 1) * 512],
                            start=(kp == 0), stop=(kp == KT // 2 - 1),
                            perf_mode=mybir.MatmulPerfMode.DoubleRow)
                for jh in range(2):
                    nc.vector.tensor_copy(
                        out=h_sb[:m, nb, jh * 512:(jh + 1) * 512], in_=pt[jh][:m, :])
                if br == 1 and nb == 0:
                    # deferred branch-0 tail: last Wf matvec + g0 activation
                    issue_gps(0, KT - 1)
                    gv0 = p_vec.tile([128, 1], f32, name="gv0", tag="gv0")
                    nc.scalar.activation(out=gv0, in_=gps[0], func=AF.Lrelu,
                                         bias=bf_sb[0], alpha=SLOPE)
                    g_vec.append(gv0)

            if br == 1:
                # head-layer psums; allocated after br1's MM1 tiles so pool
                # rotation never makes an MM1 tile wait on the (long-lived)
                # head accumulators
                xps = [ps_mm1.tile([128, 1], mybir.dt.float32,
                                   name=f"xps{mb}", tag="mm1ps")
                       for mb in range(2)]

            # ---- MM2: Z^T[j, t] = sum_s H[s, j] A^T[s, t]; fused pooling ----
            accs = p_vec.tile([128, KT, TC], f32, name=f"accs{br}", tag="accs")
            m_sb[br] = p_vec.tile([128, KT], f32, name=f"m_sb{br}", tag="m")
            gps[br] = ps_sm.tile([128, 1], mybir.dt.float32,
                                 name=f"gps{br}", tag="sps")
            for tcx in range(TC):
                if br == 1:
                    load_at(1, tcx)
                elif tcx >= 2:
                    load_at(0, tcx)
                at_t = at_sb[br][tcx]
                for j in range(KT):
                    zps = ps_mm2.tile([128, 512], mybir.dt.float32,
                                      name=f"mm2ps_{br}_{tcx}_{j}", tag="mm2ps")
                    for sp in range(ST // 2):
                        nc.tensor.matmul(
                            zps[:, :cw[tcx]],
                            lhsT=h_sb[:, 2 * sp:2 * sp + 2,
                                      j * 128:(j + 1) * 128],
                            rhs=at_t[:, 2 * sp:2 * sp + 2, :cw[tcx]],
                            start=(sp == 0), stop=(sp == ST // 2 - 1),
                            perf_mode=mybir.MatmulPerfMode.DoubleRow)
                    w = cw[tcx]
                    scr = p_scr.tile([128, 512], bf16,
                                     name=f"scr_{br}_{tcx}_{j}", tag="scr")
                    # sum_t leaky(z + bg) for this chunk, directly via Lrelu
                    nc.scalar.activation(
                        out=scr[:, :w], in_=zps[:, :w], func=AF.Lrelu,
                        bias=bgr_sb[br][:, j:j + 1], alpha=SLOPE,
                        accum_out=accs[:, j, tcx:tcx + 1])
                    if tcx == TC - 1:
                        nc.vector.tensor_reduce(
                            m_sb[br][:, j:j + 1], accs[:, j, 0:TC], AX.X, AL.add)
                        # Wf matvec one j behind, so the PE never waits on the
                        # ACT+reduce chain except for the final j
                        if j > 0:
                            issue_gps(br, j - 1)
                if br == 1 and tcx == 0:
                    # g0 half of the first head layer, off the tail
                    for mb in range(2):
                        nc.tensor.matmul(
                            xps[mb], lhsT=w1_sb[:, 0, mb * 128:(mb + 1) * 128],
                            rhs=g_vec[0], start=True, stop=False)

        # ---- tail: g1, then the head MLP with direct Lrelu ----
        issue_gps(1, KT - 1)
        gv1 = p_vec.tile([128, 1], f32, name="gv1", tag="gv1")
        nc.scalar.activation(out=gv1, in_=gps[1], func=AF.Lrelu,
                             bias=bf_sb[1], alpha=SLOPE)
        g_vec.append(gv1)

        xc1 = []
        for mb in range(2):
            nc.tensor.matmul(xps[mb],
                             lhsT=w1_sb[:, 1, mb * 128:(mb + 1) * 128],
                             rhs=g_vec[1], start=False, stop=True)
            xv = p_vec.tile([128, 1], f32, name=f"xv{mb}", tag=f"xv{mb}")
            nc.scalar.activation(out=xv, in_=xps[mb], func=AF.Lrelu,
                                 bias=b1_sb[:, mb:mb + 1], alpha=SLOPE)
            xc1.append(xv)

        x2ps = ps_mm1.tile([128, 1], mybir.dt.float32, name="x2ps", tag="mm1ps")
        for kt in range(2):
            nc.tensor.matmul(x2ps[:64], lhsT=w2_sb[:, kt, :], rhs=xc1[kt],
                             start=(kt == 0), stop=(kt == 1))
        xc2 = p_vec.tile([64, 1], f32, name="xc2", tag="xc2")
        nc.scalar.activation(out=xc2, in_=x2ps[:64], func=AF.Lrelu,
                             bias=b2_sb, alpha=SLOPE)

        ops_ = ps_mm1.tile([1, 1], mybir.dt.float32, name="ops_", tag="mm1ps")
        nc.tensor.matmul(ops_, lhsT=wo_sb, rhs=xc2, start=True, stop=True)
        osb = p_vec.tile([1, 1], f32, name="osb", tag="osb")
        nc.scalar.activation(out=osb, in_=ops_, func=AF.Sigmoid, bias=bo_sb)
        nc.scalar.dma_start(out=out_d[:, :], in_=osb)

    nc.finalize()
    return nc


def _get_nc():
    global _NC
    if _NC is None:
        _NC = _build_program()
    return _NC


def _prep_branch(x, ei):
    """Host prep for one (graph, branch): x^T fp8 and the dense normalized
    adjacency, transposed."""
    src = ei[0].astype(np.int64)
    tgt = ei[1].astype(np.int64)
    deg = (np.bincount(tgt, minlength=N) + 1).astype(np.float32)
    dinv = (1.0 / np.sqrt(deg)).astype(np.float32)
    at = np.zeros((NT, N), np.float32)
    np.add.at(at, (src, tgt), dinv[src] * dinv[tgt])
    di = np.arange(N)
    at[di, di] += dinv * dinv
    xt = np.ascontiguousarray(x.T).astype(_FP8 if FP8_MM1 else _BF16)
    return xt, at.astype(_FP8 if FP8_MM2 else _BF16)


def _make_in_maps(x1, ei1, x2, ei2, Wg1, bg1, Wf1, bf1, Wg2, bg2, Wf2, bf2,
                  W1, b1, W2, b2, Wo, bo):
    shared = {
        "wg1": np.ascontiguousarray(Wg1.astype(_FP8 if FP8_MM1 else _BF16)),
        "wg2": np.ascontiguousarray(Wg2.astype(_FP8 if FP8_MM1 else _BF16)),
        "wf1": np.ascontiguousarray((Wf1 / float(N)).astype(np.float32)),
        "wf2": np.ascontiguousarray((Wf2 / float(N)).astype(np.float32)),
        "bf1": bf1.reshape(D, 1).astype(np.float32),
        "bf2": bf2.reshape(D, 1).astype(np.float32),
        "bg1": np.ascontiguousarray(bg1.reshape(KT, 128).T.astype(np.float32)),
        "bg2": np.ascontiguousarray(bg2.reshape(KT, 128).T.astype(np.float32)),
        "w1": np.ascontiguousarray(W1.astype(np.float32)),
        "b1": np.ascontiguousarray(b1.reshape(2, 128).T.astype(np.float32)),
        "w2": np.ascontiguousarray(W2.astype(np.float32)),
        "b2": b2.reshape(64, 1).astype(np.float32),
        "wo": np.ascontiguousarray(Wo.reshape(64, 1).astype(np.float32)),
        "bo": bo.reshape(1, 1).astype(np.float32),
    }
    in_maps = []
    for b in range(B):
        m = dict(shared)
        m["xt1"], m["at1"] = _prep_branch(x1[b], ei1[b])
        m["xt2"], m["at2"] = _prep_branch(x2[b], ei2[b])
        in_maps.append(m)
    return in_maps


def kernel(**inputs):
    from concourse.bass_utils import run_bass_kernel_spmd

    nc = _get_nc()
    in_maps = _make_in_maps(**{k: np.asarray(v) for k, v in inputs.items()})
    res = run_bass_kernel_spmd(nc, in_maps, core_ids=list(range(B)))
    out = np.stack([res.results[c]["out"].reshape(1) for c in range(B)], axis=0)
    return out.astype(np.float32)


# revision 6
# speedup vs baseline: 1.1170x; 1.1170x over previous
"""Trainium2 Bass kernel for a 2-branch GCN siamese network (protein pairs).

Math per graph b (see reference):
    h  = leaky( A_norm @ (x @ Wg) + bg )        # GCNConv + LeakyReLU
    g  = leaky( mean_n(h) @ Wf + bf )
    xc = concat(g1, g2); 2-layer MLP + sigmoid -> scalar

Sharding: data-parallel over the batch of 8 graphs -> core b handles graph b
entirely (both branches + head) and emits a single scalar.

Device strategy (v2):
  - A_norm is materialized dense (2048x2000, fp8e4m3) on the host, transposed,
    with the symmetric-norm coefficients folded in.
  - MM1 (PE, fp8 DoubleRow): H[n, j] = x @ Wg, PSUM-accumulated over 4
    k-tile pairs; H is cast to fp8 on the PSUM->SBUF copy.
  - MM2 (PE, fp8 DoubleRow): Z^T[j, t] = sum_s H[s, j] * A^T[s, t],
    feature-major, 8 source-tile pairs (H pad rows zeroed).
  - pooling fused into ACT Lrelu (hw slope 0.01 == torch LeakyReLU default):
    accum_out gives sum_t leaky(z + bg) per chunk directly; a single DVE
    reduce over the 4 chunk partials yields the pooled m[j].
  - the Wf matvec (gps) is interleaved into MM2's last-chunk j loop (one j
    behind) so only one 1-col matmul remains on the tail.
  - head MLP uses plain weights + Lrelu ACTs (no (0.01W, 0.99W) pairs).
  - DMA issue is split across the SP and Activation queues; the first wg/xt
    loads are split so the first matmul starts ~9us instead of ~12us.
  - fp8 rounding washes out through the matmul contraction, 2000-node mean
    pool and the head projections: measured end-to-end rel err ~1.7e-4.
"""

import os
import sys

import numpy as np

for _p in ("/opt/trn_rl_repo", "/root/.axon_site/_ro/trn_rl_repo"):
    if os.path.isdir(_p) and _p not in sys.path:
        sys.path.insert(0, _p)

import ml_dtypes

B, N, E, F, D = 8, 2000, 64000, 1024, 128
NT = 2048          # padded node count (sources)
KT = F // 128      # 8 k-tiles over the feature dim
NB = 16            # node blocks for MM1 (15 full + one of 80)
ST = NT // 128     # 16 source tiles for MM2
TC = 4             # target chunks for MM2: widths 512,512,512,464
SLOPE = 0.01

_BF16 = ml_dtypes.bfloat16
_FP8 = ml_dtypes.float8_e4m3

FP8_MM2 = True
FP8_MM1 = True

_NC = None


def _build_program():
    import concourse.bacc as bacc
    import concourse.mybir as mybir
    import concourse.tile as tile

    f32 = mybir.dt.float32
    bf16 = mybir.dt.bfloat16
    AF = mybir.ActivationFunctionType
    AL = mybir.AluOpType
    AX = mybir.AxisListType

    nc = bacc.Bacc()

    def ein(name, shape, dt):
        return nc.dram_tensor(name, shape, dt, kind="ExternalInput")

    f8 = mybir.dt.float8e4
    hdt = f8 if FP8_MM2 else bf16
    xdt = f8 if FP8_MM1 else bf16
    xt_d = [ein("xt1", [F, N], xdt), ein("xt2", [F, N], xdt)]
    wg_d = [ein("wg1", [F, F], xdt), ein("wg2", [F, F], xdt)]
    at_d = [ein("at1", [NT, N], hdt), ein("at2", [NT, N], hdt)]
    bg_d = [ein("bg1", [128, KT], f32), ein("bg2", [128, KT], f32)]
    wf_d = [ein("wf1", [F, D], f32), ein("wf2", [F, D], f32)]
    bf_d = [ein("bf1", [D, 1], f32), ein("bf2", [D, 1], f32)]
    w1_d = ein("w1", [2 * D, 256], f32)
    b1_d = ein("b1", [128, 2], f32)
    w2_d = ein("w2", [256, 64], f32)
    b2_d = ein("b2", [64, 1], f32)
    wo_d = ein("wo", [64, 1], f32)
    bo_d = ein("bo", [1, 1], f32)
    out_d = nc.dram_tensor("out", [1, 1], f32, kind="ExternalOutput")

    cw = [512, 512, 512, 464]            # MM2 chunk widths
    c0 = [0, 512, 1024, 1536]            # chunk column offsets in A^T

    with tile.TileContext(nc) as tc, \
            tc.tile_pool(name="p_xt", bufs=2) as p_xt, \
            tc.tile_pool(name="p_wg", bufs=2) as p_wg, \
            tc.tile_pool(name="p_h", bufs=2) as p_h, \
            tc.tile_pool(name="p_at", bufs=2) as p_at, \
            tc.tile_pool(name="p_c", bufs=1) as p_c, \
            tc.tile_pool(name="p_scr", bufs=3) as p_scr, \
            tc.tile_pool(name="p_vec", bufs=2) as p_vec, \
            tc.tile_pool(name="ps_mm1", bufs=4, space="PSUM") as ps_mm1, \
            tc.tile_pool(name="ps_mm2", bufs=2, space="PSUM") as ps_mm2, \
            tc.tile_pool(name="ps_sm", bufs=2, space="PSUM") as ps_sm:

        # ================= DMA issue order is the critical path =============
        # Two issue queues: SP carries wg1 (split pair 0) + pooled consts +
        # A^T chunks + xt2; ACT carries xt1 (split pair 0) + wg2 + head
        # consts.  The first matmul needs only wg1[kp0,jh0] + xt1[kp0,n<1024].
        wg_sb = [p_wg.tile([128, KT, F], xdt, name=f"wg_sb{br}", tag="wg")
                 for br in range(2)]
        xt_sb = [p_xt.tile([128, KT, N], xdt, name=f"xt_sb{br}", tag="xt")
                 for br in range(2)]
        wgr = [wg_d[br][:, :].rearrange("(kt p) j -> p kt j", p=128)
               for br in range(2)]
        xtr = [xt_d[br][:, :].rearrange("(kt p) n -> p kt n", p=128)
               for br in range(2)]

        bgr_sb = []
        for br in range(2):
            bgr_sb.append(p_c.tile([128, KT], f32, name=f"bgr_sb{br}",
                                   tag=f"bgr{br}"))

        atr = [at_d[br][:, :].rearrange("(so p) t -> p so t", p=128)
               for br in range(2)]
        at_sb = [[p_at.tile([128, ST, 512], hdt,
                            name=f"at_sb{br}_{tcx}", tag="at")
                  for tcx in range(TC)] for br in range(2)]

        def load_at(br, tcx):
            nc.sync.dma_start(
                out=at_sb[br][tcx][:, :, :cw[tcx]],
                in_=atr[br][:, :, c0[tcx]:c0[tcx] + cw[tcx]])

        # --- all input DMAs on the sync queue: the Tile scheduler preserves
        # creation order there (the ACT queue gets reordered).  xt pairs
        # first so the last MM1 operand lands ~2us earlier; the PE then runs
        # the MM1 ramp without data gaps.
        def load_xt(br, q):
            nc.sync.dma_start(out=xt_sb[br][:, 2 * q:2 * q + 2, :],
                              in_=xtr[br][:, 2 * q:2 * q + 2, :])

        load_xt(0, 0)
        nc.sync.dma_start(out=wg_sb[0][:, 0:2, :], in_=wgr[0][:, 0:2, :])
        load_xt(0, 1)
        nc.sync.dma_start(out=wg_sb[0][:, 2:4, :], in_=wgr[0][:, 2:4, :])
        load_xt(0, 2)
        load_xt(0, 3)
        nc.sync.dma_start(out=wg_sb[0][:, 4:6, :], in_=wgr[0][:, 4:6, :])
        nc.sync.dma_start(out=wg_sb[0][:, 6:8, :], in_=wgr[0][:, 6:8, :])
        nc.sync.dma_start(out=bgr_sb[0][:], in_=bg_d[0][:, :])
        load_at(0, 0)
        load_at(0, 1)
        for h in range(2):
            nc.sync.dma_start(out=wg_sb[1][:, 4 * h:4 * h + 4, :],
                              in_=wgr[1][:, 4 * h:4 * h + 4, :])
        nc.sync.dma_start(out=bgr_sb[1][:], in_=bg_d[1][:, :])
        wf_sb, bf_sb = [], []
        for br in range(2):
            wf_t = p_c.tile([128, KT, D], f32, name=f"wf_sb{br}", tag=f"wf{br}")
            nc.sync.dma_start(
                out=wf_t[:], in_=wf_d[br][:, :].rearrange("(kt p) d -> p kt d", p=128))
            wf_sb.append(wf_t)
            bf_t = p_c.tile([D, 1], f32, name=f"bf_sb{br}", tag=f"bf{br}")
            nc.sync.dma_start(out=bf_t[:], in_=bf_d[br][:, :])
            bf_sb.append(bf_t)
        w1_sb = p_c.tile([128, 2, 256], f32, name="w1_sb", tag="w1")
        nc.sync.dma_start(
            out=w1_sb[:], in_=w1_d[:, :].rearrange("(kt p) m -> p kt m", p=128))
        b1_sb = p_c.tile([128, 2], f32, name="b1_sb", tag="b1")
        nc.sync.dma_start(out=b1_sb[:], in_=b1_d[:, :])
        w2_sb = p_c.tile([128, 2, 64], f32, name="w2_sb", tag="w2")
        nc.sync.dma_start(
            out=w2_sb[:], in_=w2_d[:, :].rearrange("(kt p) m -> p kt m", p=128))
        b2_sb = p_c.tile([64, 1], f32, name="b2_sb", tag="b2")
        nc.sync.dma_start(out=b2_sb[:], in_=b2_d[:, :])
        wo_sb = p_c.tile([64, 1], f32, name="wo_sb", tag="wo")
        nc.sync.dma_start(out=wo_sb[:], in_=wo_d[:, :])
        bo_sb = p_c.tile([1, 1], f32, name="bo_sb", tag="bo")
        nc.sync.dma_start(out=bo_sb[:], in_=bo_d[:, :])

        # warm the ACT tables (Sigmoid set; Lrelu rides in every set)
        sigwarm = p_vec.tile([1, 2], f32, name="sigwarm", tag="sigwarm")
        nc.scalar.activation(out=sigwarm[:, 0:1], in_=bo_sb, func=AF.Sigmoid)
        nc.scalar.activation(out=sigwarm[:, 1:2], in_=bo_sb, func=AF.Lrelu,
                             alpha=SLOPE)

        # ========================== compute ================================
        g_vec = []
        m_sb = [None, None]
        gps = [None, None]

        def issue_gps(br, kt):
            nc.tensor.matmul(gps[br], lhsT=wf_sb[br][:, kt, :],
                             rhs=m_sb[br][:, kt:kt + 1],
                             start=(kt == 0), stop=(kt == KT - 1))

        for br in range(2):
            if br == 1:
                for q in range(4):
                    load_xt(1, q)
            h_sb = p_h.tile([128, ST, F], hdt, name=f"h_sb{br}", tag="h")
            if FP8_MM2:
                # DoubleRow pairs contract the full 16 s-tiles, so H's pad
                # rows (2000..2047) must be zero, not garbage.
                nc.vector.memset(h_sb[64:128, 15, :], 0.0)

            # ---- MM1: H[n, j] = x @ Wg ----
            for nb in range(NB):
                n0 = nb * 128
                m = min(128, N - n0)
                pt = [ps_mm1.tile([128, 512], mybir.dt.float32,
                                  name=f"mm1ps_{br}_{nb}_{jh}", tag="mm1ps")
                      for jh in range(2)]
                for kp in range(KT // 2):
                    for jh in range(2):
                        nc.tensor.matmul(
                            pt[jh][:m, :],
                            lhsT=xt_sb[br][:, 2 * kp:2 * kp + 2, n0:n0 + m],
                            rhs=wg_sb[br][:, 2 * kp:2 * kp + 2,
                                          jh * 512:(jh + 1) * 512],
                            start=(kp == 0), stop=(kp == KT // 2 - 1),
                            perf_mode=mybir.MatmulPerfMode.DoubleRow)
                for jh in range(2):
                    nc.vector.tensor_copy(
                        out=h_sb[:m, nb, jh * 512:(jh + 1) * 512], in_=pt[jh][:m, :])
                if br == 1 and nb == 0:
                    # deferred branch-0 tail: last Wf matvec + g0 activation
                    issue_gps(0, KT - 1)
                    gv0 = p_vec.tile([128, 1], f32, name="gv0", tag="gv0")
                    nc.scalar.activation(out=gv0, in_=gps[0], func=AF.Lrelu,
                                         bias=bf_sb[0], alpha=SLOPE)
                    g_vec.append(gv0)

            if br == 1:
                # head-layer psums; allocated after br1's MM1 tiles so pool
                # rotation never makes an MM1 tile wait on the (long-lived)
                # head accumulators
                xps = [ps_mm1.tile([128, 1], mybir.dt.float32,
                                   name=f"xps{mb}", tag="mm1ps")
                       for mb in range(2)]

            # ---- MM2: Z^T[j, t] = sum_s H[s, j] A^T[s, t]; fused pooling ----
            accs = p_vec.tile([128, KT, TC], f32, name=f"accs{br}", tag="accs")
            m_sb[br] = p_vec.tile([128, KT], f32, name=f"m_sb{br}", tag="m")
            gps[br] = ps_sm.tile([128, 1], mybir.dt.float32,
                                 name=f"gps{br}", tag="sps")
            for tcx in range(TC):
                if br == 1:
                    load_at(1, tcx)
                elif tcx >= 2:
                    load_at(0, tcx)
                at_t = at_sb[br][tcx]
                for j in range(KT):
                    zps = ps_mm2.tile([128, 512], mybir.dt.float32,
                                      name=f"mm2ps_{br}_{tcx}_{j}", tag="mm2ps")
                    for sp in range(ST // 2):
                        nc.tensor.matmul(
                            zps[:, :cw[tcx]],
                            lhsT=h_sb[:, 2 * sp:2 * sp + 2,
                                      j * 128:(j + 1) * 128],
                            rhs=at_t[:, 2 * sp:2 * sp + 2, :cw[tcx]],
                            start=(sp == 0), stop=(sp == ST // 2 - 1),
                            perf_mode=mybir.MatmulPerfMode.DoubleRow)
                    w = cw[tcx]
                    scr = p_scr.tile([128, 512], bf16,
                                     name=f"scr_{br}_{tcx}_{j}", tag="scr")
                    # sum_t leaky(z + bg) for this chunk, directly via Lrelu
                    nc.scalar.activation(
                        out=scr[:, :w], in_=zps[:, :w], func=AF.Lrelu,
                        bias=bgr_sb[br][:, j:j + 1], alpha=SLOPE,
                        accum_out=accs[:, j, tcx:tcx + 1])
                    if tcx == TC - 1:
                        nc.vector.tensor_reduce(
                            m_sb[br][:, j:j + 1], accs[:, j, 0:TC], AX.X, AL.add)
                        # Wf matvec one j behind, so the PE never waits on the
                        # ACT+reduce chain except for the final j
                        if j > 0:
                            issue_gps(br, j - 1)
                if br == 1 and tcx == 0:
                    # g0 half of the first head layer, off the tail
                    for mb in range(2):
                        nc.tensor.matmul(
                            xps[mb], lhsT=w1_sb[:, 0, mb * 128:(mb + 1) * 128],
                            rhs=g_vec[0], start=True, stop=False)

        # ---- tail: g1, then the head MLP with direct Lrelu ----
        issue_gps(1, KT - 1)
        gv1 = p_vec.tile([128, 1], f32, name="gv1", tag="gv1")
        nc.scalar.activation(out=gv1, in_=gps[1], func=AF.Lrelu,
                             bias=bf_sb[1], alpha=SLOPE)
        g_vec.append(gv1)

        xc1 = []
        for mb in range(2):
            nc.tensor.matmul(xps[mb],
                             lhsT=w1_sb[:, 1, mb * 128:(mb + 1) * 128],
                             rhs=g_vec[1], start=False, stop=True)
            xv = p_vec.tile([128, 1], f32, name=f"xv{mb}", tag=f"xv{mb}")
            nc.scalar.activation(out=xv, in_=xps[mb], func=AF.Lrelu,
                                 bias=b1_sb[:, mb:mb + 1], alpha=SLOPE)
            xc1.append(xv)

        x2ps = ps_mm1.tile([128, 1], mybir.dt.float32, name="x2ps", tag="mm1ps")
        for kt in range(2):
            nc.tensor.matmul(x2ps[:64], lhsT=w2_sb[:, kt, :], rhs=xc1[kt],
                             start=(kt == 0), stop=(kt == 1))
        xc2 = p_vec.tile([64, 1], f32, name="xc2", tag="xc2")
        nc.scalar.activation(out=xc2, in_=x2ps[:64], func=AF.Lrelu,
                             bias=b2_sb, alpha=SLOPE)

        ops_ = ps_mm1.tile([1, 1], mybir.dt.float32, name="ops_", tag="mm1ps")
        nc.tensor.matmul(ops_, lhsT=wo_sb, rhs=xc2, start=True, stop=True)
        osb = p_vec.tile([1, 1], f32, name="osb", tag="osb")
        nc.scalar.activation(out=osb, in_=ops_, func=AF.Sigmoid, bias=bo_sb)
        nc.scalar.dma_start(out=out_d[:, :], in_=osb)

    nc.finalize()
    return nc


def _get_nc():
    global _NC
    if _NC is None:
        _NC = _build_program()
    return _NC


def _prep_branch(x, ei):
    """Host prep for one (graph, branch): x^T fp8 and the dense normalized
    adjacency, transposed."""
    src = ei[0].astype(np.int64)
    tgt = ei[1].astype(np.int64)
    deg = (np.bincount(tgt, minlength=N) + 1).astype(np.float32)
    dinv = (1.0 / np.sqrt(deg)).astype(np.float32)
    at = np.zeros((NT, N), np.float32)
    np.add.at(at, (src, tgt), dinv[src] * dinv[tgt])
    di = np.arange(N)
    at[di, di] += dinv * dinv
    xt = np.ascontiguousarray(x.T).astype(_FP8 if FP8_MM1 else _BF16)
    return xt, at.astype(_FP8 if FP8_MM2 else _BF16)


def _make_in_maps(x1, ei1, x2, ei2, Wg1, bg1, Wf1, bf1, Wg2, bg2, Wf2, bf2,
                  W1, b1, W2, b2, Wo, bo):
    shared = {
        "wg1": np.ascontiguousarray(Wg1.astype(_FP8 if FP8_MM1 else _BF16)),
        "wg2": np.ascontiguousarray(Wg2.astype(_FP8 if FP8_MM1 else _BF16)),
        "wf1": np.ascontiguousarray((Wf1 / float(N)).astype(np.float32)),
        "wf2": np.ascontiguousarray((Wf2 / float(N)).astype(np.float32)),
        "bf1": bf1.reshape(D, 1).astype(np.float32),
        "bf2": bf2.reshape(D, 1).astype(np.float32),
        "bg1": np.ascontiguousarray(bg1.reshape(KT, 128).T.astype(np.float32)),
        "bg2": np.ascontiguousarray(bg2.reshape(KT, 128).T.astype(np.float32)),
        "w1": np.ascontiguousarray(W1.astype(np.float32)),
        "b1": np.ascontiguousarray(b1.reshape(2, 128).T.astype(np.float32)),
        "w2": np.ascontiguousarray(W2.astype(np.float32)),
        "b2": b2.reshape(64, 1).astype(np.float32),
        "wo": np.ascontiguousarray(Wo.reshape(64, 1).astype(np.float32)),
        "bo": bo.reshape(1, 1).astype(np.float32),
    }
    in_maps = []
    for b in range(B):
        m = dict(shared)
        m["xt1"], m["at1"] = _prep_branch(x1[b], ei1[b])
        m["xt2"], m["at2"] = _prep_branch(x2[b], ei2[b])
        in_maps.append(m)
    return in_maps


def kernel(**inputs):
    from concourse.bass_utils import run_bass_kernel_spmd

    nc = _get_nc()
    in_maps = _make_in_maps(**{k: np.asarray(v) for k, v in inputs.items()})
    res = run_bass_kernel_spmd(nc, in_maps, core_ids=list(range(B)))
    out = np.stack([res.results[c]["out"].reshape(1) for c in range(B)], axis=0)
    return out.astype(np.float32)
